# revision 1
# baseline (speedup 1.0000x reference)
"""Trainium2 Bass kernel for nn_ConcatNonLinear (LightGCN propagation + GAT + MLP).

Self-contained: accepts FULL inputs, shards across 8 NeuronCores internally,
returns the FULL [4096] output.

Math (matches reference):
  E0 = [E_u; E_v]                         [7000, 64]
  E_l = A_tilde @ E_{l-1},  l = 1..3      (dense, row-sharded + AllGather)
  h = [E0|E1|E2|E3]                       [7000, 256]
  Wh = h @ W + bW; s1 = Wh@a1+ba1; s2 = Wh@a2+ba2
  The attention is exactly sparse: adj has at most 2*4096 nonzeros, and
  softmax rows with no edges are never selected by the output, so the
  softmax/attn@Wh stage runs in the edge domain: per-edge dedup weights
  (duplicate (u,v) pairs count once, matching .at[].set), one-hot matmuls
  for the segment sums, a global-s2max shift for exp safety (cancels in
  the softmax ratio exactly).
  out = MLP([H[users] | H[items+4000]])   [4096]
"""
import numpy as np
from contextlib import ExitStack

import concourse.bass as bass
import concourse.mybir as mybir
import concourse.tile as tile
from concourse import bacc
from concourse import bass_isa
from concourse.bass_utils import run_bass_kernel_spmd
from concourse.masks import make_identity

NCORES = 8
N = 7000
NU = 4000
NV = 3000
NP = 7040            # node dim padded to 55*128
T = 55               # k-tiles of 128 over NP
KT = 5               # k-tiles loaded per A DMA
RT = 25              # k-tiles of A kept resident in SBUF across layers
K = 64
F = 256
B = 4096
RS = 875             # node rows per core
RSP = 876            # padded even (fp32r moving width must be even)
HJ = 438             # half of RSP
BS = 512             # batch rows per core
ALPHA = 0.2
NODE_CH = 7          # ceil(896/128) one-hot M chunks
SC = 12              # edge chunks (of 128) per super-chunk

f32 = mybir.dt.float32
f32r = mybir.dt.float32r
i32 = mybir.dt.int32

_cache: dict = {}


def _build(EC: int, reps: int = 1, do_prop: bool = True, tail: bool = True,
           prop_layers: int = 3, fake_gathers: bool = False, ag_off: bool = False):
    """Build + compile the SPMD Bass program (shared by all 8 cores)."""
    NCH = EC // 128
    nc = bacc.Bacc("TRN2", target_bir_lowering=False, debug=False,
                   num_devices=NCORES)

    def din(name, shape, dt=f32):
        return nc.dram_tensor(name, shape, dt, kind="ExternalInput").ap()

    atp_d = din("atp", [NP, RSP], f32r)    # A[rows_c, :].T  (k-major), padded
    e0_d = din("e0", [NP, K], f32r)        # full E0, zero-padded rows
    e0t_d = din("e0t", [K, RSP], f32r)     # E0[rows_c].T, padded
    w_d = din("w_in", [F, F], f32r)
    a12_d = din("a12_in", [F, 2], f32r)
    bw_d = din("bw_in", [128, 2])
    ba12_d = din("ba12_in", [2, 1])
    w1_d = din("w1_in", [2 * F, 128], f32r)
    b1_d = din("b1_in", [128, 1])
    w2_d = din("w2_in", [128, K], f32r)
    b2_d = din("b2_in", [K, 1])
    w3_d = din("w3_in", [K, 1], f32r)
    b3_d = din("b3_in", [1, 1])
    edr_d = din("edr", [EC, 1], i32)       # directed edge row node (global id)
    edc_d = din("edc", [EC, 1], i32)       # directed edge col node (global id)
    edl_d = din("edl", [EC, 1])            # row node local id (float)
    edlt_d = din("edlt", [1, EC], f32r)    # same, transposed (for ohT trick)
    edw_d = din("edw", [EC, 1])            # dedup weight (0 => dead pad edge)
    usr_d = din("usr", [BS, 1], i32)       # this core's batch users
    itm_d = din("itm", [BS, 1], i32)       # this core's batch items + NU
    out_d = nc.dram_tensor("out", [1, BS], f32, kind="ExternalOutput").ap()

    RG = [list(range(NCORES))]

    def do_ag(nc_, in_ap, out_ap, nrows):
        if ag_off:
            nc_.scalar.dma_start(out_ap.tensor.ap()[0:nrows], in_ap)
        else:
            nc_.gpsimd.collective_compute(
                "AllGather", mybir.AluOpType.bypass, replica_groups=RG,
                ins=[in_ap], outs=[out_ap])

    with tile.TileContext(nc) as tc:
      for _rep in range(reps):
        with ExitStack() as ctx:
            const = ctx.enter_context(tc.tile_pool(name="const", bufs=1))
            sb = ctx.enter_context(tc.tile_pool(name="sb", bufs=1))
            dr = ctx.enter_context(tc.tile_pool(name="dr", bufs=1, space="DRAM"))

            ident = const.tile([128, 128], f32)
            make_identity(nc, ident)
            identr = const.tile([128, 128], f32r)
            nc.vector.tensor_copy(identr[:], ident[:])
            iota_f = const.tile([128, NODE_CH * 128], f32)
            iota_i = const.tile([128, NODE_CH * 128], i32)
            nc.gpsimd.iota(iota_i[:], pattern=[[1, NODE_CH * 128]], base=0,
                           channel_multiplier=0)
            nc.vector.tensor_copy(iota_f[:], iota_i[:])
            iotap_i = const.tile([128, 1], i32)
            nc.gpsimd.iota(iotap_i[:], pattern=[[1, 1]], base=0,
                           channel_multiplier=1)
            iotap_f = const.tile([128, 1], f32)
            nc.vector.tensor_copy(iotap_f[:], iotap_i[:])
            ones_r = const.tile([1, 128], f32r)
            nc.vector.memset(ones_r[:].bitcast(f32), 1.0)

            # ---------------- params to SBUF (all HWDGE) ----------------
            w_sb = const.tile([128, 2 * F], f32r)       # [128,(kb,f')]
            nc.scalar.dma_start(
                w_sb[:].rearrange("p (kb f) -> p kb f", kb=2),
                w_d.rearrange("(kb p) f -> p kb f", p=128))
            a12_sb = const.tile([128, 2 * 2], f32r)
            nc.scalar.dma_start(
                a12_sb[:].rearrange("p (kb f) -> p kb f", kb=2),
                a12_d.rearrange("(kb p) f -> p kb f", p=128))
            bw_sb = const.tile([128, 2], f32)
            nc.scalar.dma_start(bw_sb[:], bw_d[:])
            ba12_sb = const.tile([2, 1], f32)
            nc.scalar.dma_start(ba12_sb[:], ba12_d[:])
            w1_sb = const.tile([128, 4 * 128], f32r)
            nc.scalar.dma_start(
                w1_sb[:].rearrange("p (kb f) -> p kb f", kb=4),
                w1_d.rearrange("(kb p) f -> p kb f", p=128))
            b1_sb = const.tile([128, 1], f32)
            nc.scalar.dma_start(b1_sb[:], b1_d[:])
            w2_sb = const.tile([128, K], f32r)
            nc.scalar.dma_start(w2_sb[:], w2_d[:])
            b2_sb = const.tile([K, 1], f32)
            nc.scalar.dma_start(b2_sb[:], b2_d[:])
            w3_sb = const.tile([K, 1], f32r)
            nc.scalar.dma_start(w3_sb[:], w3_d[:])
            b3_sb = const.tile([1, 1], f32)
            nc.scalar.dma_start(b3_sb[:], b3_d[:])

            # h^T, built up layer by layer: [128, (half, RSP)]
            hT = sb.tile([128, 2 * RSP], f32r, tag="hT")
            hT3 = hT[:].rearrange("p (half r) -> p half r", half=2)
            nc.scalar.dma_start(hT3[0:64, 0, :], e0t_d[:])  # E0^T

            # ---------------- propagation: 3 layers ----------------
            if do_prop:
             with tc.tile_pool(name="apool", bufs=3) as apool, \
                  tc.tile_pool(name="arpool", bufs=1) as arpool, \
                  tc.tile_pool(name="epool", bufs=2) as epool, \
                  tc.tile_pool(name="prop_ps", bufs=2, space="PSUM") as pps, \
                  tc.tile_pool(name="prop_tp", bufs=2, space="PSUM") as tps, \
                  tc.tile_pool(name="prop_sb", bufs=2) as psb:
                # E0 natural tiles [128, (t, f)]
                e_cur = epool.tile([128, T * K], f32r, tag="ecur")
                nc.scalar.dma_start(
                    e_cur[:].rearrange("p (t f) -> p t f", t=T),
                    e0_d.rearrange("(t p) f -> p t f", p=128))
                # resident prefix of A^T (k-tiles 0..RT), loaded once
                ares = arpool.tile([128, RT * RSP], f32r, tag="ares")
                for tch in range(RT // KT):
                    nc.sync.dma_start(
                        ares[:, tch * KT * RSP:(tch + 1) * KT * RSP]
                        .rearrange("p (t r) -> p t r", t=KT),
                        atp_d.rearrange("(tc t p) r -> tc p t r",
                                        t=KT, p=128)[tch])
                for layer in range(1, prop_layers + 1):
                    pj = [pps.tile([K, HJ], f32, name=f"pj{j}", tag=f"pj{j}")
                          for j in range(2)]
                    for tch in range(T // KT):
                        if tch * KT < RT:
                            a_t = ares[:, tch * KT * RSP:(tch + 1) * KT * RSP]
                        else:
                            a_tt = apool.tile([128, KT * RSP], f32r, tag="a_t")
                            nc.sync.dma_start(
                                a_tt[:].rearrange("p (t r) -> p t r", t=KT),
                                atp_d.rearrange("(tc t p) r -> tc p t r",
                                                t=KT, p=128)[tch])
                            a_t = a_tt[:]
                        for tt in range(KT):
                            t = tch * KT + tt
                            for j in range(2):
                                nc.tensor.matmul(
                                    pj[j][:],
                                    e_cur[:, t * K:(t + 1) * K],
                                    a_t[:, tt * RSP + j * HJ:
                                        tt * RSP + (j + 1) * HJ],
                                    start=(t == 0), stop=(t == T - 1))
                    # epilogue: E_l^T into hT; if l<3, AllGather natural E_l
                    half, poff = ((0, 64), (1, 0), (1, 64))[layer - 1]
                    for j in range(2):
                        nc.vector.tensor_copy(
                            hT3[poff:poff + 64, half, j * HJ:(j + 1) * HJ],
                            pj[j][:])
                    if layer < 3:
                        nat = psb.tile([128, 7 * K], f32r, tag="nat")
                        for c in range(7):
                            w = 128 if c < 6 else RS - 6 * 128
                            tp = tps.tile([128, K], f32r, tag="tp")
                            nc.tensor.transpose(
                                tp[:w, :],
                                hT3[poff:poff + 64, half, c * 128:c * 128 + w],
                                identr[poff:poff + 64, poff:poff + 64])
                            nc.vector.tensor_copy(
                                nat[:w, c * K:(c + 1) * K], tp[:w, :])
                        ag_in = dr.tile([RS, K], f32r, tag=f"agi{layer}")
                        nc.scalar.dma_start(
                            ag_in[0:768, :].rearrange(
                                "(c p) f -> p c f", p=128),
                            nat[:].rearrange(
                                "p (c f) -> p c f", c=7)[:, 0:6, :])
                        nc.scalar.dma_start(
                            ag_in[768:RS, :], nat[:107, 6 * K:7 * K])
                        ag_out = dr.tile([N, K], f32r, tag=f"ago{layer}",
                                         addr_space="Shared")
                        do_ag(nc, ag_in[:], ag_out[:], RS)
                        e_cur = epool.tile([128, T * K], f32r, tag="ecur")
                        nc.vector.memset(
                            e_cur[:, 54 * K:].bitcast(f32), 0.0)
                        for h3 in range(3):
                            nc.scalar.dma_start(
                                e_cur[:, h3 * 18 * K:(h3 + 1) * 18 * K]
                                .rearrange("p (t f) -> p t f", t=18),
                                ag_out[h3 * 2304:(h3 + 1) * 2304, :]
                                .rearrange("(t p) f -> p t f", p=128))
                        nc.scalar.dma_start(
                            e_cur[0:88, 54 * K:], ag_out[6912:N, :])

            # ---------------- Wh, s1, s2, table AllGather ----------------
            if not tail:
                nc.scalar.dma_start(out_d[:], hT[0:1, 0:BS].bitcast(f32))
                continue
            whT_f = sb.tile([128, 2 * RSP], f32, tag="whT_f")
            whT_f3 = whT_f[:].rearrange("p (mj r) -> p mj r", mj=2)
            whT_r = sb.tile([128, 2 * RSP], f32r, tag="whT_r")
            whT_r3 = whT_r[:].rearrange("p (mj r) -> p mj r", mj=2)
            w_sb3 = w_sb[:].rearrange("p (kb f) -> p kb f", kb=2)
            s2max_b = sb.tile([128, 1], f32, tag="s2max_b")
            tbl = dr.tile([N, 260], f32, tag="tbl", addr_space="Shared")
            with tc.tile_pool(name="wh_ps", bufs=2, space="PSUM") as wps, \
                  tc.tile_pool(name="wh_tp", bufs=1, space="PSUM") as wtp, \
                  tc.tile_pool(name="wh_sb", bufs=2) as wsb:
                for mj in range(2):
                    for j in range(2):
                        ps = wps.tile([128, HJ], f32, tag="wh")
                        for kb in range(2):
                            nc.tensor.matmul(
                                ps[:],
                                w_sb3[:, kb, mj * 128:(mj + 1) * 128],
                                hT3[:, kb, j * HJ:(j + 1) * HJ],
                                start=(kb == 0), stop=(kb == 1))
                        nc.scalar.activation(
                            whT_f3[:, mj, j * HJ:(j + 1) * HJ], ps[:],
                            mybir.ActivationFunctionType.Identity,
                            bias=bw_sb[:, mj:mj + 1])
                for mj in range(2):
                    nc.vector.tensor_copy(whT_r3[:, mj, :], whT_f3[:, mj, :])
                # s = [s1 s2]^T: [2, RSP]
                sT = wsb.tile([2, RSP], f32, tag="sT")
                a12_sb3 = a12_sb[:].rearrange("p (kb f) -> p kb f", kb=2)
                for j in range(2):
                    ps = wps.tile([2, HJ], f32, tag="sps")
                    for kb in range(2):
                        nc.tensor.matmul(
                            ps[:], a12_sb3[:, kb, :],
                            whT_r3[:, kb, j * HJ:(j + 1) * HJ],
                            start=(kb == 0), stop=(kb == 1))
                    nc.scalar.activation(
                        sT[:, j * HJ:(j + 1) * HJ], ps[:],
                        mybir.ActivationFunctionType.Identity,
                        bias=ba12_sb[:, 0:1])
                # local [s1max, s2max] -> tiny AllGather -> global s2 max
                smax_l = wsb.tile([2, 1], f32, tag="smax_l")
                nc.vector.tensor_reduce(
                    smax_l[:], sT[:, 0:RS], axis=mybir.AxisListType.X,
                    op=mybir.AluOpType.max)
                tpm = wtp.tile([1, 2], f32, tag="tpm")
                nc.tensor.transpose(tpm[:], smax_l[:], ident[0:2, 0:2])
                smax_sb = wsb.tile([1, 2], f32, tag="smax_sb")
                nc.vector.tensor_copy(smax_sb[:], tpm[:])
                smax_bc = wsb.tile([128, 2], f32, tag="smax_bc")
                nc.gpsimd.partition_broadcast(smax_bc[:], smax_sb[0:1, :])

                # natural table shard [875, 260] = [Wh | s1 | s2 | 0 0]
                tbl_nat = wsb.tile([128, 7 * 260], f32, tag="tbl_nat")
                tn3 = tbl_nat[:].rearrange("p (c f) -> p c f", c=7)
                nc.vector.memset(tbl_nat[:], 0.0)
                for c in range(7):
                    w = 128 if c < 6 else RS - 6 * 128
                    for mj in range(2):
                        tp = wtp.tile([128, 128], f32, tag="tpw")
                        nc.tensor.transpose(
                            tp[:w, :], whT_f3[:, mj, c * 128:c * 128 + w],
                            ident[:])
                        nc.vector.tensor_copy(
                            tn3[:w, c, mj * 128:(mj + 1) * 128], tp[:w, :])
                    tps2 = wtp.tile([128, 2], f32, tag="tps2")
                    nc.tensor.transpose(
                        tps2[:w, :], sT[:, c * 128:c * 128 + w],
                        ident[0:2, 0:2])
                    nc.vector.tensor_copy(tn3[:w, c, 256:258], tps2[:w, :])
                    nc.vector.tensor_copy(tn3[:w, c, 258:260], smax_bc[:w, :])
                s1loc_r = sb.tile([128, NODE_CH], f32r, tag="s1loc_r")
                nc.vector.tensor_copy(s1loc_r[:], tn3[:, :, 256])
                tbl_in = dr.tile([RS, 260], f32, tag="tbl_in")
                nc.scalar.dma_start(
                    tbl_in[0:768, :].rearrange("(c p) f -> p c f", p=128),
                    tn3[:, 0:6, :])
                nc.scalar.dma_start(
                    tbl_in[768:RS, :], tn3[:107, 6, :])
                do_ag(nc, tbl_in[:], tbl[:], RS)
                smax_all = wsb.tile([NCORES, 2], f32, tag="smax_all")
                nc.scalar.dma_start(
                    smax_all[:],
                    tbl[:].rearrange("(c p) f -> c p f", c=NCORES)
                    [:, 0, 258:260])
                smax_red = wsb.tile([NCORES, 2], f32, tag="smax_red")
                nc.gpsimd.partition_all_reduce(
                    smax_red[:], smax_all[:], NCORES, bass_isa.ReduceOp.max)
                nc.gpsimd.partition_broadcast(
                    s2max_b[:], smax_red[0:1, 1:2])

            # ---------------- sparse attention (edge domain, batched) -------
            hfull = dr.tile([N, F], f32, tag="hfull", addr_space="Shared")
            n_sc = (NCH + SC - 1) // SC
            NQ = (EC + 511) // 512
            with tc.tile_pool(name="ed_sb", bufs=1) as esb, \
                  tc.tile_pool(name="ed_sb2", bufs=2) as esb2:
                # per-edge s1[row] without gathers: transposed-onehot matmuls
                # (scoped PSUM pool; runs concurrently with the tbl AllGather)
                s1r = esb.tile([128, NCH], f32, tag="s1r")
                with tc.tile_pool(name="ed_ps2", bufs=2,
                                  space="PSUM") as eps2:
                    rlocT = esb.tile([1, EC], f32r, tag="rlocT")
                    nc.scalar.dma_start(rlocT[:], edlt_d[:])
                    rlocB = esb.tile([128, EC], f32, tag="rlocB")
                    for q in range(NQ):
                        qs = slice(q * 512, min((q + 1) * 512, EC))
                        qw = qs.stop - qs.start
                        pb = eps2.tile([128, 512], f32, tag="pbq")
                        nc.tensor.matmul(pb[:, :qw], ones_r[:], rlocT[:, qs],
                                         start=True, stop=True)
                        nc.vector.tensor_copy(rlocB[:, qs], pb[:, :qw])
                    s1rowT = esb.tile([1, EC], f32, tag="s1rowT")
                    ohTm = esb.tile([128, EC], f32r, tag="ohTm")
                    for q in range(NQ):
                        qs = slice(q * 512, min((q + 1) * 512, EC))
                        qw = qs.stop - qs.start
                        ps1 = eps2.tile([1, 512], f32, tag="ps1q")
                        for m in range(NODE_CH):
                            nc.vector.tensor_scalar(
                                ohTm[:, qs], rlocB[:, qs],
                                float(-128 * m), iotap_f[:, :1],
                                op0=mybir.AluOpType.add,
                                op1=mybir.AluOpType.is_equal)
                            nc.tensor.matmul(
                                ps1[:, :qw], s1loc_r[:, m:m + 1], ohTm[:, qs],
                                start=(m == 0), stop=(m == NODE_CH - 1))
                        nc.vector.tensor_copy(s1rowT[:, qs], ps1[:, :qw])
                    s1r_ps = eps2.tile([128, NCH], f32, tag="s1r_ps")
                    for ch in range(NCH):
                        nc.tensor.transpose(
                            s1r_ps[:, ch:ch + 1],
                            s1rowT[0:1, ch * 128:(ch + 1) * 128],
                            ident[0:1, 0:1])
                    nc.vector.tensor_copy(s1r[:], s1r_ps[:])
                s1r3 = s1r[:].rearrange("p (ch x) -> p ch x", x=1)
                with tc.tile_pool(name="ed_ps", bufs=1,
                                  space="PSUM") as eps:
                    acc = [eps.tile([128, 258], f32, name=f"acc{m}", tag=f"acc{m}")
                           for m in range(NODE_CH)]
                    for scc in range(n_sc):
                        ch0 = scc * SC
                        nch = min(SC, NCH - ch0)
                        ne = nch * 128
                        esl = slice(ch0 * 128, ch0 * 128 + ne)
                        cidx = esb2.tile([128, SC], i32, tag="cidx")
                        rloc = esb2.tile([128, SC], f32, tag="rloc")
                        wdw = esb2.tile([128, SC], f32, tag="wdw")
                        for tl, src in ((cidx, edc_d),
                                        (rloc, edl_d), (wdw, edw_d)):
                            nc.scalar.dma_start(
                                tl[:, :nch].rearrange("p (ch x) -> p ch x", x=1),
                                src[esl, :].rearrange("(ch p) x -> p ch x", p=128))
                        G = esb2.tile([128, SC * 260], f32, tag="G")
                        G3 = G[:].rearrange("p (ch f) -> p ch f", ch=SC)
                        for ch in range(nch):
                            if fake_gathers:
                                nc.scalar.dma_start(
                                    G3[:, ch, :], tbl[ch * 128:(ch + 1) * 128, :])
                                continue
                            nc.gpsimd.indirect_dma_start(
                                out=G3[:, ch, :], out_offset=None, in_=tbl[:],
                                in_offset=bass.IndirectOffsetOnAxis(
                                    ap=cidx[:, ch:ch + 1], axis=0))
                        # wide per-edge math over [128, nch]
                        te = esb2.tile([128, SC], f32, tag="te")
                        te3 = te[:].rearrange("p (ch x) -> p ch x", x=1)
                        nc.vector.tensor_tensor(
                            out=te3[:, :nch, :],
                            in0=s1r3[:, ch0:ch0 + nch, :],
                            in1=G3[:, :nch, 257:258], op=mybir.AluOpType.add)
                        ev = esb2.tile([128, SC], f32, tag="ev")
                        nc.vector.tensor_scalar(
                            ev[:, :nch], te[:, :nch], ALPHA, None,
                            op0=mybir.AluOpType.mult)
                        nc.vector.tensor_tensor(
                            out=ev[:, :nch], in0=ev[:, :nch], in1=te[:, :nch],
                            op=mybir.AluOpType.max)
                        t2 = esb2.tile([128, SC], f32, tag="t2")
                        t23 = t2[:].rearrange("p (ch x) -> p ch x", x=1)
                        nc.vector.tensor_scalar(
                            t23[:, :nch, :], s1r3[:, ch0:ch0 + nch, :],
                            s2max_b[:, :1], None, op0=mybir.AluOpType.add)
                        cv = esb2.tile([128, SC], f32, tag="cv")
                        nc.vector.tensor_scalar(
                            cv[:, :nch], t2[:, :nch], ALPHA, None,
                            op0=mybir.AluOpType.mult)
                        nc.vector.tensor_tensor(
                            out=cv[:, :nch], in0=cv[:, :nch], in1=t2[:, :nch],
                            op=mybir.AluOpType.max)
                        nc.vector.tensor_tensor(
                            out=ev[:, :nch], in0=ev[:, :nch], in1=cv[:, :nch],
                            op=mybir.AluOpType.subtract)
                        ex = esb2.tile([128, SC], f32, tag="ex")
                        nc.scalar.activation(
                            ex[:, :nch], ev[:, :nch],
                            mybir.ActivationFunctionType.Exp)
                        exw = esb2.tile([128, SC], f32, tag="exw")
                        nc.vector.tensor_tensor(
                            out=exw[:, :nch], in0=ex[:, :nch], in1=wdw[:, :nch],
                            op=mybir.AluOpType.mult)
                        exw3 = exw[:].rearrange("p (ch x) -> p ch x", x=1)
                        S = esb2.tile([128, SC * 258], f32r, tag="S")
                        S3 = S[:].rearrange("p (ch f) -> p ch f", ch=SC)
                        nc.vector.tensor_tensor(
                            out=S3[:, :nch, 0:256], in0=G3[:, :nch, 0:256],
                            in1=exw3[:, :nch, :].to_broadcast([128, nch, 256]),
                            op=mybir.AluOpType.mult)
                        nc.vector.tensor_copy(
                            S3[:, :nch, 256:258],
                            exw3[:, :nch, :].to_broadcast([128, nch, 2]))
                        oh = esb.tile([128, SC * NODE_CH * 128], f32r, tag="oh")
                        oh3 = oh[:].rearrange("p (ch f) -> p ch f", ch=SC)
                        rloc3 = rloc[:].rearrange("p (ch x) -> p ch x", x=1)
                        iota3 = iota_f[:].rearrange("p (x f) -> p x f", x=1)
                        nc.vector.tensor_tensor(
                            out=oh3[:, :nch, :],
                            in0=rloc3[:, :nch, :].to_broadcast(
                                [128, nch, NODE_CH * 128]),
                            in1=iota3[:, :, :].to_broadcast(
                                [128, nch, NODE_CH * 128]),
                            op=mybir.AluOpType.is_equal)
                        for ch in range(nch):
                            gch = ch0 + ch
                            for m in range(NODE_CH):
                                nc.tensor.matmul(
                                    acc[m][:],
                                    oh3[:, ch, m * 128:(m + 1) * 128],
                                    S3[:, ch, :],
                                    start=(gch == 0), stop=(gch == NCH - 1))
                    # H rows = acc[:, :256] / (acc[:, 256] + eps)
                    hnat = esb.tile([128, 7 * F], f32, tag="hnat")
                    hn3 = hnat[:].rearrange("p (c f) -> p c f", c=7)
                    for m in range(7):
                        w = 128 if m < 6 else RS - 6 * 128
                        z = esb2.tile([128, 1], f32, tag="z")
                        nc.vector.tensor_scalar(
                            z[:w, :], acc[m][:w, 256:257], 1e-30, None,
                            op0=mybir.AluOpType.add)
                        rz = esb2.tile([128, 1], f32, tag="rz")
                        nc.vector.reciprocal(rz[:w, :], z[:w, :])
                        nc.vector.tensor_scalar(
                            hn3[:w, m, :], acc[m][:w, 0:256], rz[:w, :1], None,
                            op0=mybir.AluOpType.mult)
                    hin = dr.tile([RS, F], f32, tag="hin")
                    nc.scalar.dma_start(
                        hin[0:768, :].rearrange("(c p) f -> p c f", p=128),
                        hn3[:, 0:6, :])
                    nc.scalar.dma_start(
                        hin[768:RS, :], hn3[:107, 6, :])
                    do_ag(nc, hin[:], hfull[:], RS)

            # ---------------- MLP on this core's 512 batch rows -------------
            with tc.tile_pool(name="ml_ps", bufs=1, space="PSUM") as mps, \
                  tc.tile_pool(name="ml_tp", bufs=2, space="PSUM") as mtp_p, \
                  tc.tile_pool(name="ml_sb", bufs=3) as msb:
                uidx = msb.tile([128, 4], i32, tag="uidx")
                vidx = msb.tile([128, 4], i32, tag="vidx")
                nc.scalar.dma_start(
                    uidx[:].rearrange("p (ch x) -> p ch x", x=1),
                    usr_d.rearrange("(ch p) x -> p ch x", p=128))
                nc.scalar.dma_start(
                    vidx[:].rearrange("p (ch x) -> p ch x", x=1),
                    itm_d.rearrange("(ch p) x -> p ch x", p=128))
                xT = msb.tile([128, 4 * BS], f32r, tag="xT")
                xT3 = xT[:].rearrange("p (kb b) -> p kb b", kb=4)
                for t in range(4):
                    gu = msb.tile([128, F], f32, tag="gu")
                    gv = msb.tile([128, F], f32, tag="gv")
                    if fake_gathers:
                        nc.scalar.dma_start(
                            gu[:], hfull[t * 128:(t + 1) * 128, :])
                        nc.scalar.dma_start(
                            gv[:], hfull[t * 128:(t + 1) * 128, :])
                    else:
                        nc.gpsimd.indirect_dma_start(
                            out=gu[:], out_offset=None, in_=hfull[:],
                            in_offset=bass.IndirectOffsetOnAxis(
                                ap=uidx[:, t:t + 1], axis=0))
                        nc.gpsimd.indirect_dma_start(
                            out=gv[:], out_offset=None, in_=hfull[:],
                            in_offset=bass.IndirectOffsetOnAxis(
                                ap=vidx[:, t:t + 1], axis=0))
                    for fc in range(2):
                        tp = mtp_p.tile([128, 128], f32, tag="mtp")
                        nc.tensor.transpose(
                            tp[:], gu[:, fc * 128:(fc + 1) * 128], ident[:])
                        nc.vector.tensor_copy(
                            xT3[:, fc, t * 128:(t + 1) * 128], tp[:])
                    for fc in range(2):
                        tp = mtp_p.tile([128, 128], f32, tag="mtp")
                        nc.tensor.transpose(
                            tp[:], gv[:, fc * 128:(fc + 1) * 128], ident[:])
                        nc.vector.tensor_copy(
                            xT3[:, 2 + fc, t * 128:(t + 1) * 128], tp[:])
                w1_sb3 = w1_sb[:].rearrange("p (kb f) -> p kb f", kb=4)
                y1p = mps.tile([128, BS], f32, tag="y1p")
                for kb in range(4):
                    nc.tensor.matmul(
                        y1p[:], w1_sb3[:, kb, :], xT3[:, kb, :],
                        start=(kb == 0), stop=(kb == 3))
                y1 = msb.tile([128, BS], f32r, tag="y1")
                nc.scalar.activation(
                    y1[:], y1p[:], mybir.ActivationFunctionType.Gelu,
                    bias=b1_sb[:, 0:1])
                y2p = mps.tile([K, BS], f32, tag="y2p")
                nc.tensor.matmul(y2p[:], w2_sb[:], y1[:], start=True, stop=True)
                y2 = msb.tile([K, BS], f32r, tag="y2")
                nc.scalar.activation(
                    y2[:], y2p[:], mybir.ActivationFunctionType.Gelu,
                    bias=b2_sb[:, 0:1])
                y3p = mps.tile([1, BS], f32, tag="y3p")
                nc.tensor.matmul(y3p[:], w3_sb[:], y2[:], start=True, stop=True)
                yo = msb.tile([1, BS], f32, tag="yo")
                nc.scalar.activation(
                    yo[:], y3p[:], mybir.ActivationFunctionType.Identity,
                    bias=b3_sb[:, 0:1])
                nc.scalar.dma_start(out_d[:], yo[:])

    nc.compile()
    return nc


def _prep(users, items, A_tilde, E_u, E_v, W, bW, a1, ba1, a2, ba2,
          W1, b1, W2, b2, W3, b3):
    users = np.asarray(users).astype(np.int64).ravel()
    items = np.asarray(items).astype(np.int64).ravel()
    A = np.asarray(A_tilde, dtype=np.float32)
    E0 = np.concatenate([np.asarray(E_u, np.float32),
                         np.asarray(E_v, np.float32)], axis=0)
    E0p = np.zeros((NP, K), np.float32)
    E0p[:N] = E0

    # directed edges with dedup weights
    wn = items + NU
    pk = users * NV + items
    uniq, inv, cnt = np.unique(pk, return_inverse=True, return_counts=True)
    dw = (1.0 / cnt[inv]).astype(np.float32)
    rows = np.concatenate([users, wn])
    cols = np.concatenate([wn, users])
    ws = np.concatenate([dw, dw])
    shard = rows // RS
    max_cnt = int(np.bincount(shard, minlength=NCORES).max())
    EC = max(1536, ((max_cnt + 127) // 128) * 128)

    bw2 = np.ascontiguousarray(
        np.asarray(bW, np.float32).reshape(2, 128).T)
    in_maps = []
    for c in range(NCORES):
        r0 = c * RS
        sel = shard == c
        rc, cc, wc = rows[sel], cols[sel], ws[sel]
        order = np.argsort(rc, kind="stable")
        rc, cc, wc = rc[order], cc[order], wc[order]
        npad = EC - len(rc)
        edr = np.concatenate([rc, np.full(npad, r0)]).astype(np.int32)
        edc = np.concatenate([cc, np.zeros(npad)]).astype(np.int32)
        edl = (edr.astype(np.float32) - np.float32(r0))
        edw = np.concatenate([wc, np.zeros(npad, np.float32)])

        atp = np.zeros((NP, RSP), np.float32)
        atp[:N, :RS] = np.ascontiguousarray(A[r0:r0 + RS, :].T)
        e0t = np.zeros((K, RSP), np.float32)
        e0t[:, :RS] = E0[r0:r0 + RS, :].T

        in_maps.append({
            "atp": atp, "e0": E0p, "e0t": e0t,
            "w_in": np.asarray(W, np.float32),
            "a12_in": np.ascontiguousarray(
                np.stack([np.asarray(a1, np.float32),
                          np.asarray(a2, np.float32)], axis=1)),
            "bw_in": bw2,
            "ba12_in": np.array([[np.float32(np.asarray(ba1).ravel()[0])],
                                 [np.float32(np.asarray(ba2).ravel()[0])]],
                                np.float32),
            "w1_in": np.asarray(W1, np.float32),
            "b1_in": np.asarray(b1, np.float32).reshape(128, 1),
            "w2_in": np.asarray(W2, np.float32),
            "b2_in": np.asarray(b2, np.float32).reshape(K, 1),
            "w3_in": np.asarray(W3, np.float32),
            "b3_in": np.asarray(b3, np.float32).reshape(1, 1),
            "edr": edr.reshape(EC, 1), "edc": edc.reshape(EC, 1),
            "edl": edl.reshape(EC, 1), "edlt": edl.reshape(1, EC).copy(),
            "edw": edw.reshape(EC, 1),
            "usr": users[c * BS:(c + 1) * BS].astype(np.int32).reshape(BS, 1),
            "itm": (items[c * BS:(c + 1) * BS] + NU).astype(
                np.int32).reshape(BS, 1),
        })
    return EC, in_maps


def kernel(**inputs) -> np.ndarray:
    EC, in_maps = _prep(**inputs)
    if EC not in _cache:
        _cache[EC] = _build(EC)
    nc = _cache[EC]
    res = run_bass_kernel_spmd(nc, in_maps, core_ids=list(range(NCORES)))
    out = np.concatenate([r["out"].reshape(-1) for r in res.results])
    return out.astype(np.float32)



# revision 8
# speedup vs baseline: 1.5398x; 1.5398x over previous
"""Trainium2 Bass kernel for nn_ConcatNonLinear (LightGCN propagation + GAT + MLP).

Self-contained: accepts FULL inputs, shards across 8 NeuronCores internally,
returns the FULL [4096] output.

Math (matches reference):
  E0 = [E_u; E_v]                         [7000, 64]
  E_l = A_tilde @ E_{l-1},  l = 1..3      (dense, row-sharded + AllGather)
  h = [E0|E1|E2|E3]                       [7000, 256]
  Wh = h @ W + bW; s1 = Wh@a1+ba1; s2 = Wh@a2+ba2
  The attention is exactly sparse: adj has at most 2*4096 nonzeros, and
  softmax rows with no edges are never selected by the output, so the
  softmax/attn@Wh stage runs in the edge domain: per-edge dedup weights
  (duplicate (u,v) pairs count once, matching .at[].set), one-hot matmuls
  for the segment sums, a global-s2max shift for exp safety (cancels in
  the softmax ratio exactly).
  out = MLP([H[users] | H[items+4000]])   [4096]
"""
import numpy as np
import ml_dtypes
from contextlib import ExitStack

import concourse.bass as bass
import concourse.mybir as mybir
import concourse.tile as tile
from concourse import bacc
from concourse import bass_isa
from concourse.bass_utils import run_bass_kernel_spmd
from concourse.masks import make_identity

NCORES = 8
N = 7000
NU = 4000
NV = 3000
NP = 7040            # node dim padded to 55*128
T = 55               # k-tiles of 128 over NP
KT = 5               # k-tiles loaded per A DMA chunk
K = 64
F = 256
B = 4096
RS = 875             # node rows per core
RSP = 876            # padded even (fp32r moving width must be even)
HJ = 438             # half of RSP
BS = 512             # batch rows per core
ALPHA = 0.2
NODE_CH = 7          # ceil(896/128) one-hot M chunks
SC = 12              # edge chunks (of 128) per super-chunk

f32 = mybir.dt.float32
f32r = mybir.dt.float32r
bf16 = mybir.dt.bfloat16
i32 = mybir.dt.int32

_cache: dict = {}


def _build(EC: int, reps: int = 1, do_prop: bool = True, tail: bool = True,
           prop_layers: int = 3, fake_gathers: bool = False, ag_off: bool = False):
    """Build + compile the SPMD Bass program (shared by all 8 cores)."""
    NCH = EC // 128
    nc = bacc.Bacc("TRN2", target_bir_lowering=False, debug=False,
                   num_devices=NCORES)

    def din(name, shape, dt=f32):
        return nc.dram_tensor(name, shape, dt, kind="ExternalInput").ap()

    atp_d = din("atp", [NP, RSP], bf16)    # A[rows_c, :].T  (k-major), padded
    e0_d = din("e0", [NP, K], bf16)        # full E0, zero-padded rows
    e0t_d = din("e0t", [K, RSP], f32r)     # E0[rows_c].T, padded
    w_d = din("w_in", [F, F], f32r)
    a12_d = din("a12_in", [F, 2], f32r)
    bw_d = din("bw_in", [128, 2])
    ba12_d = din("ba12_in", [2, 1])
    w1_d = din("w1_in", [2 * F, 128], f32r)
    b1_d = din("b1_in", [128, 1])
    w2_d = din("w2_in", [128, K], f32r)
    b2_d = din("b2_in", [K, 1])
    w3_d = din("w3_in", [K, 1], f32r)
    b3_d = din("b3_in", [1, 1])
    edr_d = din("edr", [EC, 1], i32)       # directed edge row node (global id)
    edc_d = din("edc", [EC, 1], i32)       # directed edge col node (global id)
    edl_d = din("edl", [EC, 1])            # row node local id (float)
    edlt_d = din("edlt", [1, EC], f32r)    # same, transposed (for ohT trick)
    edw_d = din("edw", [EC, 1])            # dedup weight (0 => dead pad edge)
    usr_d = din("usr", [BS, 1], i32)       # this core's batch users
    itm_d = din("itm", [BS, 1], i32)       # this core's batch items + NU
    out_d = nc.dram_tensor("out", [1, BS], f32, kind="ExternalOutput").ap()

    RG = [list(range(NCORES))]

    def do_ag(nc_, in_ap, out_ap, nrows):
        if ag_off:
            nc_.scalar.dma_start(out_ap.tensor.ap()[0:nrows], in_ap)
        else:
            nc_.gpsimd.collective_compute(
                "AllGather", mybir.AluOpType.bypass, replica_groups=RG,
                ins=[in_ap], outs=[out_ap])

    with tile.TileContext(nc) as tc:
      for _rep in range(reps):
        with ExitStack() as ctx:
            const = ctx.enter_context(tc.tile_pool(name="const", bufs=1))
            sb = ctx.enter_context(tc.tile_pool(name="sb", bufs=1))
            dr = ctx.enter_context(tc.tile_pool(name="dr", bufs=1, space="DRAM"))

            ident = const.tile([128, 128], f32)
            make_identity(nc, ident)
            identr = const.tile([128, 128], f32r)
            nc.vector.tensor_copy(identr[:], ident[:])
            iota_f = const.tile([128, NODE_CH * 128], f32)
            iota_i = const.tile([128, NODE_CH * 128], i32)
            nc.gpsimd.iota(iota_i[:], pattern=[[1, NODE_CH * 128]], base=0,
                           channel_multiplier=0)
            nc.vector.tensor_copy(iota_f[:], iota_i[:])
            iotap_i = const.tile([128, 1], i32)
            nc.gpsimd.iota(iotap_i[:], pattern=[[1, 1]], base=0,
                           channel_multiplier=1)
            iotap_f = const.tile([128, 1], f32)
            nc.vector.tensor_copy(iotap_f[:], iotap_i[:])
            ones_r = const.tile([1, 128], f32r)
            nc.vector.memset(ones_r[:].bitcast(f32), 1.0)

            # ---------------- params to SBUF (all HWDGE) ----------------
            w_sb = const.tile([128, 2 * F], f32r)       # [128,(kb,f')]
            nc.scalar.dma_start(
                w_sb[:].rearrange("p (kb f) -> p kb f", kb=2),
                w_d.rearrange("(kb p) f -> p kb f", p=128))
            a12_sb = const.tile([128, 2 * 2], f32r)
            nc.scalar.dma_start(
                a12_sb[:].rearrange("p (kb f) -> p kb f", kb=2),
                a12_d.rearrange("(kb p) f -> p kb f", p=128))
            bw_sb = const.tile([128, 2], f32)
            nc.scalar.dma_start(bw_sb[:], bw_d[:])
            ba12_sb = const.tile([2, 1], f32)
            nc.scalar.dma_start(ba12_sb[:], ba12_d[:])
            w1_sb = const.tile([128, 4 * 128], f32r)
            nc.scalar.dma_start(
                w1_sb[:].rearrange("p (kb f) -> p kb f", kb=4),
                w1_d.rearrange("(kb p) f -> p kb f", p=128))
            b1_sb = const.tile([128, 1], f32)
            nc.scalar.dma_start(b1_sb[:], b1_d[:])
            w2_sb = const.tile([128, K], f32r)
            nc.scalar.dma_start(w2_sb[:], w2_d[:])
            b2_sb = const.tile([K, 1], f32)
            nc.scalar.dma_start(b2_sb[:], b2_d[:])
            w3_sb = const.tile([K, 1], f32r)
            nc.scalar.dma_start(w3_sb[:], w3_d[:])
            b3_sb = const.tile([1, 1], f32)
            nc.scalar.dma_start(b3_sb[:], b3_d[:])

            # h^T, built up layer by layer: [128, (half, RSP)]
            hT = sb.tile([128, 2 * RSP], f32r, tag="hT")
            hT3 = hT[:].rearrange("p (half r) -> p half r", half=2)
            nc.scalar.dma_start(hT3[0:64, 0, :], e0t_d[:])  # E0^T

            # ---------------- propagation: 3 layers ----------------
            if do_prop:
             with tc.tile_pool(name="arpool", bufs=1) as arpool, \
                  tc.tile_pool(name="epool", bufs=2) as epool, \
                  tc.tile_pool(name="prop_ps", bufs=2, space="PSUM") as pps, \
                  tc.tile_pool(name="prop_tp", bufs=2, space="PSUM") as tps, \
                  tc.tile_pool(name="prop_sb", bufs=2) as psb:
                # E0 natural tiles [128, (t, f)]
                e_cur = epool.tile([128, T * K], bf16, tag="ecur")
                nc.scalar.dma_start(
                    e_cur[:].rearrange("p (t f) -> p t f", t=T),
                    e0_d.rearrange("(t p) f -> p t f", p=128))
                # ALL of A^T resident in SBUF (bf16), loaded once per rep
                ares = arpool.tile([128, T * RSP], bf16, tag="ares")
                for tch in range(T // KT):
                    nc.sync.dma_start(
                        ares[:, tch * KT * RSP:(tch + 1) * KT * RSP]
                        .rearrange("p (t r) -> p t r", t=KT),
                        atp_d.rearrange("(tc t p) r -> tc p t r",
                                        t=KT, p=128)[tch])
                for layer in range(1, prop_layers + 1):
                    pj = [pps.tile([K, HJ], f32, name=f"pj{j}", tag=f"pj{j}")
                          for j in range(2)]
                    for t in range(T):
                        for j in range(2):
                            nc.tensor.matmul(
                                pj[j][:],
                                e_cur[:, t * K:(t + 1) * K],
                                ares[:, t * RSP + j * HJ:
                                     t * RSP + (j + 1) * HJ],
                                start=(t == 0), stop=(t == T - 1))
                    # epilogue: E_l^T into hT; if l<3, AllGather natural E_l
                    half, poff = ((0, 64), (1, 0), (1, 64))[layer - 1]
                    for j in range(2):
                        nc.vector.tensor_copy(
                            hT3[poff:poff + 64, half, j * HJ:(j + 1) * HJ],
                            pj[j][:])
                    if layer < 3:
                        nat = psb.tile([128, 7 * K], bf16, tag="nat")
                        for c in range(7):
                            w = 128 if c < 6 else RS - 6 * 128
                            tp = tps.tile([128, K], f32r, tag="tp")
                            nc.tensor.transpose(
                                tp[:w, :],
                                hT3[poff:poff + 64, half, c * 128:c * 128 + w],
                                identr[poff:poff + 64, poff:poff + 64])
                            nc.vector.tensor_copy(
                                nat[:w, c * K:(c + 1) * K], tp[:w, :])
                        ag_in = dr.tile([RS, K], bf16, tag=f"agi{layer}")
                        nc.scalar.dma_start(
                            ag_in[0:768, :].rearrange(
                                "(c p) f -> p c f", p=128),
                            nat[:].rearrange(
                                "p (c f) -> p c f", c=7)[:, 0:6, :])
                        nc.scalar.dma_start(
                            ag_in[768:RS, :], nat[:107, 6 * K:7 * K])
                        ag_out = dr.tile([N, K], bf16, tag=f"ago{layer}",
                                         addr_space="Shared")
                        do_ag(nc, ag_in[:], ag_out[:], RS)
                        e_cur = epool.tile([128, T * K], bf16, tag="ecur")
                        nc.vector.memset(e_cur[:, 54 * K:], 0.0)
                        for h3 in range(3):
                            nc.scalar.dma_start(
                                e_cur[:, h3 * 18 * K:(h3 + 1) * 18 * K]
                                .rearrange("p (t f) -> p t f", t=18),
                                ag_out[h3 * 2304:(h3 + 1) * 2304, :]
                                .rearrange("(t p) f -> p t f", p=128))
                        nc.scalar.dma_start(
                            e_cur[0:88, 54 * K:], ag_out[6912:N, :])

            # ---------------- Wh, s1, s2, table AllGather ----------------
            if not tail:
                nc.scalar.dma_start(out_d[:], hT[0:1, 0:BS].bitcast(f32))
                continue
            whT_f = sb.tile([128, 2 * RSP], f32, tag="whT_f")
            whT_f3 = whT_f[:].rearrange("p (mj r) -> p mj r", mj=2)
            whT_r = sb.tile([128, 2 * RSP], f32r, tag="whT_r")
            whT_r3 = whT_r[:].rearrange("p (mj r) -> p mj r", mj=2)
            w_sb3 = w_sb[:].rearrange("p (kb f) -> p kb f", kb=2)
            s2max_b = sb.tile([128, 1], f32, tag="s2max_b")
            tbl = dr.tile([N, 260], f32, tag="tbl", addr_space="Shared")
            with tc.tile_pool(name="wh_ps", bufs=2, space="PSUM") as wps, \
                  tc.tile_pool(name="wh_tp", bufs=1, space="PSUM") as wtp, \
                  tc.tile_pool(name="wh_sb", bufs=2) as wsb:
                for mj in range(2):
                    for j in range(2):
                        ps = wps.tile([128, HJ], f32, tag="wh")
                        for kb in range(2):
                            nc.tensor.matmul(
                                ps[:],
                                w_sb3[:, kb, mj * 128:(mj + 1) * 128],
                                hT3[:, kb, j * HJ:(j + 1) * HJ],
                                start=(kb == 0), stop=(kb == 1))
                        nc.scalar.activation(
                            whT_f3[:, mj, j * HJ:(j + 1) * HJ], ps[:],
                            mybir.ActivationFunctionType.Identity,
                            bias=bw_sb[:, mj:mj + 1])
                for mj in range(2):
                    nc.vector.tensor_copy(whT_r3[:, mj, :], whT_f3[:, mj, :])
                # s = [s1 s2]^T: [2, RSP]
                sT = wsb.tile([2, RSP], f32, tag="sT")
                a12_sb3 = a12_sb[:].rearrange("p (kb f) -> p kb f", kb=2)
                for j in range(2):
                    ps = wps.tile([2, HJ], f32, tag="sps")
                    for kb in range(2):
                        nc.tensor.matmul(
                            ps[:], a12_sb3[:, kb, :],
                            whT_r3[:, kb, j * HJ:(j + 1) * HJ],
                            start=(kb == 0), stop=(kb == 1))
                    nc.scalar.activation(
                        sT[:, j * HJ:(j + 1) * HJ], ps[:],
                        mybir.ActivationFunctionType.Identity,
                        bias=ba12_sb[:, 0:1])
                # local [s1max, s2max] -> tiny AllGather -> global s2 max
                smax_l = wsb.tile([2, 1], f32, tag="smax_l")
                nc.vector.tensor_reduce(
                    smax_l[:], sT[:, 0:RS], axis=mybir.AxisListType.X,
                    op=mybir.AluOpType.max)
                tpm = wtp.tile([1, 2], f32, tag="tpm")
                nc.tensor.transpose(tpm[:], smax_l[:], ident[0:2, 0:2])
                smax_sb = wsb.tile([1, 2], f32, tag="smax_sb")
                nc.vector.tensor_copy(smax_sb[:], tpm[:])
                smax_bc = wsb.tile([128, 2], f32, tag="smax_bc")
                nc.gpsimd.partition_broadcast(smax_bc[:], smax_sb[0:1, :])

                # natural table shard [875, 260] = [Wh | s1 | s2 | 0 0]
                tbl_nat = wsb.tile([128, 7 * 260], f32, tag="tbl_nat")
                tn3 = tbl_nat[:].rearrange("p (c f) -> p c f", c=7)
                nc.vector.memset(tbl_nat[:], 0.0)
                for c in range(7):
                    w = 128 if c < 6 else RS - 6 * 128
                    for mj in range(2):
                        tp = wtp.tile([128, 128], f32, tag="tpw")
                        nc.tensor.transpose(
                            tp[:w, :], whT_f3[:, mj, c * 128:c * 128 + w],
                            ident[:])
                        nc.vector.tensor_copy(
                            tn3[:w, c, mj * 128:(mj + 1) * 128], tp[:w, :])
                    tps2 = wtp.tile([128, 2], f32, tag="tps2")
                    nc.tensor.transpose(
                        tps2[:w, :], sT[:, c * 128:c * 128 + w],
                        ident[0:2, 0:2])
                    nc.vector.tensor_copy(tn3[:w, c, 256:258], tps2[:w, :])
                    nc.vector.tensor_copy(tn3[:w, c, 258:260], smax_bc[:w, :])
                s1loc_r = sb.tile([128, NODE_CH], f32r, tag="s1loc_r")
                nc.vector.tensor_copy(s1loc_r[:], tn3[:, :, 256])
                tbl_in = dr.tile([RS, 260], f32, tag="tbl_in")
                nc.scalar.dma_start(
                    tbl_in[0:768, :].rearrange("(c p) f -> p c f", p=128),
                    tn3[:, 0:6, :])
                nc.scalar.dma_start(
                    tbl_in[768:RS, :], tn3[:107, 6, :])
                do_ag(nc, tbl_in[:], tbl[:], RS)
                smax_all = wsb.tile([NCORES, 2], f32, tag="smax_all")
                nc.scalar.dma_start(
                    smax_all[:],
                    tbl[:].rearrange("(c p) f -> c p f", c=NCORES)
                    [:, 0, 258:260])
                smax_red = wsb.tile([NCORES, 2], f32, tag="smax_red")
                nc.gpsimd.partition_all_reduce(
                    smax_red[:], smax_all[:], NCORES, bass_isa.ReduceOp.max)
                nc.gpsimd.partition_broadcast(
                    s2max_b[:], smax_red[0:1, 1:2])

            # ---------------- sparse attention (edge domain, batched) -------
            hfull = dr.tile([N, F], f32, tag="hfull", addr_space="Shared")
            n_sc = (NCH + SC - 1) // SC
            NQ = (EC + 511) // 512
            with tc.tile_pool(name="ed_sb", bufs=1) as esb, \
                  tc.tile_pool(name="ed_sb2", bufs=2) as esb2:
                # per-edge s1[row] without gathers: transposed-onehot matmuls
                # (scoped PSUM pool; runs concurrently with the tbl AllGather)
                s1r = esb.tile([128, NCH], f32, tag="s1r")
                with tc.tile_pool(name="ed_ps2", bufs=2,
                                  space="PSUM") as eps2:
                    rlocT = esb.tile([1, EC], f32r, tag="rlocT")
                    nc.scalar.dma_start(rlocT[:], edlt_d[:])
                    rlocB = esb.tile([128, EC], f32, tag="rlocB")
                    for q in range(NQ):
                        qs = slice(q * 512, min((q + 1) * 512, EC))
                        qw = qs.stop - qs.start
                        pb = eps2.tile([128, 512], f32, tag="pbq")
                        nc.tensor.matmul(pb[:, :qw], ones_r[:], rlocT[:, qs],
                                         start=True, stop=True)
                        nc.vector.tensor_copy(rlocB[:, qs], pb[:, :qw])
                    s1rowT = esb.tile([1, EC], f32, tag="s1rowT")
                    ohTm = esb.tile([128, EC], f32r, tag="ohTm")
                    for q in range(NQ):
                        qs = slice(q * 512, min((q + 1) * 512, EC))
                        qw = qs.stop - qs.start
                        ps1 = eps2.tile([1, 512], f32, tag="ps1q")
                        for m in range(NODE_CH):
                            nc.vector.tensor_scalar(
                                ohTm[:, qs], rlocB[:, qs],
                                float(-128 * m), iotap_f[:, :1],
                                op0=mybir.AluOpType.add,
                                op1=mybir.AluOpType.is_equal)
                            nc.tensor.matmul(
                                ps1[:, :qw], s1loc_r[:, m:m + 1], ohTm[:, qs],
                                start=(m == 0), stop=(m == NODE_CH - 1))
                        nc.vector.tensor_copy(s1rowT[:, qs], ps1[:, :qw])
                    s1r_ps = eps2.tile([128, NCH], f32, tag="s1r_ps")
                    for ch in range(NCH):
                        nc.tensor.transpose(
                            s1r_ps[:, ch:ch + 1],
                            s1rowT[0:1, ch * 128:(ch + 1) * 128],
                            ident[0:1, 0:1])
                    nc.vector.tensor_copy(s1r[:], s1r_ps[:])
                s1r3 = s1r[:].rearrange("p (ch x) -> p ch x", x=1)
                with tc.tile_pool(name="ed_ps", bufs=1,
                                  space="PSUM") as eps:
                    acc = [eps.tile([128, 258], f32, name=f"acc{m}", tag=f"acc{m}")
                           for m in range(NODE_CH)]
                    for scc in range(n_sc):
                        ch0 = scc * SC
                        nch = min(SC, NCH - ch0)
                        ne = nch * 128
                        esl = slice(ch0 * 128, ch0 * 128 + ne)
                        cidx = esb2.tile([128, SC], i32, tag="cidx")
                        rloc = esb2.tile([128, SC], f32, tag="rloc")
                        wdw = esb2.tile([128, SC], f32, tag="wdw")
                        for tl, src in ((cidx, edc_d),
                                        (rloc, edl_d), (wdw, edw_d)):
                            nc.scalar.dma_start(
                                tl[:, :nch].rearrange("p (ch x) -> p ch x", x=1),
                                src[esl, :].rearrange("(ch p) x -> p ch x", p=128))
                        G = esb2.tile([128, SC * 260], f32, tag="G")
                        G3 = G[:].rearrange("p (ch f) -> p ch f", ch=SC)
                        for ch in range(nch):
                            if fake_gathers:
                                nc.scalar.dma_start(
                                    G3[:, ch, :], tbl[ch * 128:(ch + 1) * 128, :])
                                continue
                            nc.gpsimd.indirect_dma_start(
                                out=G3[:, ch, :], out_offset=None, in_=tbl[:],
                                in_offset=bass.IndirectOffsetOnAxis(
                                    ap=cidx[:, ch:ch + 1], axis=0))
                        # wide per-edge math over [128, nch]
                        te = esb2.tile([128, SC], f32, tag="te")
                        te3 = te[:].rearrange("p (ch x) -> p ch x", x=1)
                        nc.vector.tensor_tensor(
                            out=te3[:, :nch, :],
                            in0=s1r3[:, ch0:ch0 + nch, :],
                            in1=G3[:, :nch, 257:258], op=mybir.AluOpType.add)
                        ev = esb2.tile([128, SC], f32, tag="ev")
                        nc.vector.tensor_scalar(
                            ev[:, :nch], te[:, :nch], ALPHA, None,
                            op0=mybir.AluOpType.mult)
                        nc.vector.tensor_tensor(
                            out=ev[:, :nch], in0=ev[:, :nch], in1=te[:, :nch],
                            op=mybir.AluOpType.max)
                        t2 = esb2.tile([128, SC], f32, tag="t2")
                        t23 = t2[:].rearrange("p (ch x) -> p ch x", x=1)
                        nc.vector.tensor_scalar(
                            t23[:, :nch, :], s1r3[:, ch0:ch0 + nch, :],
                            s2max_b[:, :1], None, op0=mybir.AluOpType.add)
                        cv = esb2.tile([128, SC], f32, tag="cv")
                        nc.vector.tensor_scalar(
                            cv[:, :nch], t2[:, :nch], ALPHA, None,
                            op0=mybir.AluOpType.mult)
                        nc.vector.tensor_tensor(
                            out=cv[:, :nch], in0=cv[:, :nch], in1=t2[:, :nch],
                            op=mybir.AluOpType.max)
                        nc.vector.tensor_tensor(
                            out=ev[:, :nch], in0=ev[:, :nch], in1=cv[:, :nch],
                            op=mybir.AluOpType.subtract)
                        ex = esb2.tile([128, SC], f32, tag="ex")
                        nc.scalar.activation(
                            ex[:, :nch], ev[:, :nch],
                            mybir.ActivationFunctionType.Exp)
                        exw = esb2.tile([128, SC], f32, tag="exw")
                        nc.vector.tensor_tensor(
                            out=exw[:, :nch], in0=ex[:, :nch], in1=wdw[:, :nch],
                            op=mybir.AluOpType.mult)
                        exw3 = exw[:].rearrange("p (ch x) -> p ch x", x=1)
                        S = esb2.tile([128, SC * 258], f32r, tag="S")
                        S3 = S[:].rearrange("p (ch f) -> p ch f", ch=SC)
                        nc.vector.tensor_tensor(
                            out=S3[:, :nch, 0:256], in0=G3[:, :nch, 0:256],
                            in1=exw3[:, :nch, :].to_broadcast([128, nch, 256]),
                            op=mybir.AluOpType.mult)
                        nc.vector.tensor_copy(
                            S3[:, :nch, 256:258],
                            exw3[:, :nch, :].to_broadcast([128, nch, 2]))
                        oh = esb.tile([128, SC * NODE_CH * 128], f32r, tag="oh")
                        oh3 = oh[:].rearrange("p (ch f) -> p ch f", ch=SC)
                        rloc3 = rloc[:].rearrange("p (ch x) -> p ch x", x=1)
                        iota3 = iota_f[:].rearrange("p (x f) -> p x f", x=1)
                        nc.vector.tensor_tensor(
                            out=oh3[:, :nch, :],
                            in0=rloc3[:, :nch, :].to_broadcast(
                                [128, nch, NODE_CH * 128]),
                            in1=iota3[:, :, :].to_broadcast(
                                [128, nch, NODE_CH * 128]),
                            op=mybir.AluOpType.is_equal)
                        for ch in range(nch):
                            gch = ch0 + ch
                            for m in range(NODE_CH):
                                nc.tensor.matmul(
                                    acc[m][:],
                                    oh3[:, ch, m * 128:(m + 1) * 128],
                                    S3[:, ch, :],
                                    start=(gch == 0), stop=(gch == NCH - 1))
                    # H rows = acc[:, :256] / (acc[:, 256] + eps)
                    hnat = esb.tile([128, 7 * F], f32, tag="hnat")
                    hn3 = hnat[:].rearrange("p (c f) -> p c f", c=7)
                    for m in range(7):
                        w = 128 if m < 6 else RS - 6 * 128
                        z = esb2.tile([128, 1], f32, tag="z")
                        nc.vector.tensor_scalar(
                            z[:w, :], acc[m][:w, 256:257], 1e-30, None,
                            op0=mybir.AluOpType.add)
                        rz = esb2.tile([128, 1], f32, tag="rz")
                        nc.vector.reciprocal(rz[:w, :], z[:w, :])
                        nc.vector.tensor_scalar(
                            hn3[:w, m, :], acc[m][:w, 0:256], rz[:w, :1], None,
                            op0=mybir.AluOpType.mult)
                    hin = dr.tile([RS, F], f32, tag="hin")
                    nc.scalar.dma_start(
                        hin[0:768, :].rearrange("(c p) f -> p c f", p=128),
                        hn3[:, 0:6, :])
                    nc.scalar.dma_start(
                        hin[768:RS, :], hn3[:107, 6, :])
                    do_ag(nc, hin[:], hfull[:], RS)

            # ---------------- MLP on this core's 512 batch rows -------------
            with tc.tile_pool(name="ml_ps", bufs=1, space="PSUM") as mps, \
                  tc.tile_pool(name="ml_tp", bufs=2, space="PSUM") as mtp_p, \
                  tc.tile_pool(name="ml_sb", bufs=3) as msb:
                uidx = msb.tile([128, 4], i32, tag="uidx")
                vidx = msb.tile([128, 4], i32, tag="vidx")
                nc.scalar.dma_start(
                    uidx[:].rearrange("p (ch x) -> p ch x", x=1),
                    usr_d.rearrange("(ch p) x -> p ch x", p=128))
                nc.scalar.dma_start(
                    vidx[:].rearrange("p (ch x) -> p ch x", x=1),
                    itm_d.rearrange("(ch p) x -> p ch x", p=128))
                xT = msb.tile([128, 4 * BS], f32r, tag="xT")
                xT3 = xT[:].rearrange("p (kb b) -> p kb b", kb=4)
                for t in range(4):
                    gu = msb.tile([128, F], f32, tag="gu")
                    gv = msb.tile([128, F], f32, tag="gv")
                    if fake_gathers:
                        nc.scalar.dma_start(
                            gu[:], hfull[t * 128:(t + 1) * 128, :])
                        nc.scalar.dma_start(
                            gv[:], hfull[t * 128:(t + 1) * 128, :])
                    else:
                        nc.gpsimd.indirect_dma_start(
                            out=gu[:], out_offset=None, in_=hfull[:],
                            in_offset=bass.IndirectOffsetOnAxis(
                                ap=uidx[:, t:t + 1], axis=0))
                        nc.gpsimd.indirect_dma_start(
                            out=gv[:], out_offset=None, in_=hfull[:],
                            in_offset=bass.IndirectOffsetOnAxis(
                                ap=vidx[:, t:t + 1], axis=0))
                    for fc in range(2):
                        tp = mtp_p.tile([128, 128], f32, tag="mtp")
                        nc.tensor.transpose(
                            tp[:], gu[:, fc * 128:(fc + 1) * 128], ident[:])
                        nc.vector.tensor_copy(
                            xT3[:, fc, t * 128:(t + 1) * 128], tp[:])
                    for fc in range(2):
                        tp = mtp_p.tile([128, 128], f32, tag="mtp")
                        nc.tensor.transpose(
                            tp[:], gv[:, fc * 128:(fc + 1) * 128], ident[:])
                        nc.vector.tensor_copy(
                            xT3[:, 2 + fc, t * 128:(t + 1) * 128], tp[:])
                w1_sb3 = w1_sb[:].rearrange("p (kb f) -> p kb f", kb=4)
                y1p = mps.tile([128, BS], f32, tag="y1p")
                for kb in range(4):
                    nc.tensor.matmul(
                        y1p[:], w1_sb3[:, kb, :], xT3[:, kb, :],
                        start=(kb == 0), stop=(kb == 3))
                y1 = msb.tile([128, BS], f32r, tag="y1")
                nc.scalar.activation(
                    y1[:], y1p[:], mybir.ActivationFunctionType.Gelu,
                    bias=b1_sb[:, 0:1])
                y2p = mps.tile([K, BS], f32, tag="y2p")
                nc.tensor.matmul(y2p[:], w2_sb[:], y1[:], start=True, stop=True)
                y2 = msb.tile([K, BS], f32r, tag="y2")
                nc.scalar.activation(
                    y2[:], y2p[:], mybir.ActivationFunctionType.Gelu,
                    bias=b2_sb[:, 0:1])
                y3p = mps.tile([1, BS], f32, tag="y3p")
                nc.tensor.matmul(y3p[:], w3_sb[:], y2[:], start=True, stop=True)
                yo = msb.tile([1, BS], f32, tag="yo")
                nc.scalar.activation(
                    yo[:], y3p[:], mybir.ActivationFunctionType.Identity,
                    bias=b3_sb[:, 0:1])
                nc.scalar.dma_start(out_d[:], yo[:])

    nc.compile()
    return nc


def _prep(users, items, A_tilde, E_u, E_v, W, bW, a1, ba1, a2, ba2,
          W1, b1, W2, b2, W3, b3):
    users = np.asarray(users).astype(np.int64).ravel()
    items = np.asarray(items).astype(np.int64).ravel()
    A = np.asarray(A_tilde, dtype=np.float32)
    E0 = np.concatenate([np.asarray(E_u, np.float32),
                         np.asarray(E_v, np.float32)], axis=0)
    E0p = np.zeros((NP, K), ml_dtypes.bfloat16)
    E0p[:N] = E0.astype(ml_dtypes.bfloat16)

    # directed edges with dedup weights
    wn = items + NU
    pk = users * NV + items
    uniq, inv, cnt = np.unique(pk, return_inverse=True, return_counts=True)
    dw = (1.0 / cnt[inv]).astype(np.float32)
    rows = np.concatenate([users, wn])
    cols = np.concatenate([wn, users])
    ws = np.concatenate([dw, dw])
    shard = rows // RS
    max_cnt = int(np.bincount(shard, minlength=NCORES).max())
    EC = max(1536, ((max_cnt + 127) // 128) * 128)

    bw2 = np.ascontiguousarray(
        np.asarray(bW, np.float32).reshape(2, 128).T)
    in_maps = []
    for c in range(NCORES):
        r0 = c * RS
        sel = shard == c
        rc, cc, wc = rows[sel], cols[sel], ws[sel]
        order = np.argsort(rc, kind="stable")
        rc, cc, wc = rc[order], cc[order], wc[order]
        npad = EC - len(rc)
        edr = np.concatenate([rc, np.full(npad, r0)]).astype(np.int32)
        edc = np.concatenate([cc, np.zeros(npad)]).astype(np.int32)
        edl = (edr.astype(np.float32) - np.float32(r0))
        edw = np.concatenate([wc, np.zeros(npad, np.float32)])

        atp = np.zeros((NP, RSP), ml_dtypes.bfloat16)
        atp[:N, :RS] = A[r0:r0 + RS, :].T.astype(ml_dtypes.bfloat16)
        e0t = np.zeros((K, RSP), np.float32)
        e0t[:, :RS] = E0[r0:r0 + RS, :].T

        in_maps.append({
            "atp": atp, "e0": E0p, "e0t": e0t,
            "w_in": np.asarray(W, np.float32),
            "a12_in": np.ascontiguousarray(
                np.stack([np.asarray(a1, np.float32),
                          np.asarray(a2, np.float32)], axis=1)),
            "bw_in": bw2,
            "ba12_in": np.array([[np.float32(np.asarray(ba1).ravel()[0])],
                                 [np.float32(np.asarray(ba2).ravel()[0])]],
                                np.float32),
            "w1_in": np.asarray(W1, np.float32),
            "b1_in": np.asarray(b1, np.float32).reshape(128, 1),
            "w2_in": np.asarray(W2, np.float32),
            "b2_in": np.asarray(b2, np.float32).reshape(K, 1),
            "w3_in": np.asarray(W3, np.float32),
            "b3_in": np.asarray(b3, np.float32).reshape(1, 1),
            "edr": edr.reshape(EC, 1), "edc": edc.reshape(EC, 1),
            "edl": edl.reshape(EC, 1), "edlt": edl.reshape(1, EC).copy(),
            "edw": edw.reshape(EC, 1),
            "usr": users[c * BS:(c + 1) * BS].astype(np.int32).reshape(BS, 1),
            "itm": (items[c * BS:(c + 1) * BS] + NU).astype(
                np.int32).reshape(BS, 1),
        })
    return EC, in_maps


def kernel(**inputs) -> np.ndarray:
    EC, in_maps = _prep(**inputs)
    if EC not in _cache:
        _cache[EC] = _build(EC)
    nc = _cache[EC]
    res = run_bass_kernel_spmd(nc, in_maps, core_ids=list(range(NCORES)))
    out = np.concatenate([r["out"].reshape(-1) for r in res.results])
    return out.astype(np.float32)

